# revision 31
# baseline (speedup 1.0000x reference)
"""Trainium2 Bass kernel for a Qwen2-VL vision transformer block.

Strategy: 8-way sequence-parallel across NeuronCores. Each core owns a
256-row shard of the 2048-token sequence and the full weights (bf16).
K/V for the full sequence are exchanged with a single fp8(e4m3)
AllGather; K/V/Q and the softmax weights stay fp8 through the attention
matmuls (fp32 PSUM), which halves the collective payload and the
gathered-operand SBUF footprint at ~1e-4 extra relative error. K and V
are projected and published per 4-head group so the collective triggers
as early as possible.

Layout notes:
  - Projections produce activations in natural [seq, feat] layout; the
    PE transpose (identity matmul) produces the [feat, seq] operands
    that later matmuls need as stationary input.
  - V is published pre-augmented: rows are [v(80) | 1] per head, packed
    head-major, so the softmax denominator is accumulated by the extra
    column during the attn x V matmul and the post-gather loads are
    fully contiguous (one DMA per 128-row rank block).
  - Attention computes scores^T [key, query] per head; the per-query
    normalizer 1/Z is broadcast across partitions with a rank-1
    outer-product matmul.
  - LayerNorm affine params and the quick-gelu 1.702 scale are folded
    into the weights on the host, which is exact in fp32.
"""

import sys

import numpy as np

for _p in ("/opt/trn_rl_repo",):
    if _p not in sys.path:
        sys.path.insert(0, _p)

import ml_dtypes  # noqa: E402


BF = ml_dtypes.bfloat16

B, S, H = 1, 2048, 1280
NH, HD = 16, 80
MLP = 5120
EPS = 1e-6
NCORES = 8
SL = S // NCORES            # 256 sequence rows per core
SB = SL // 128              # 2 partition blocks per core
HC = H // 128               # 10 contraction chunks over H
MC = MLP // 128             # 40 blocks over the MLP dim
KB = S // 128               # 16 key blocks over the full sequence
NCOLS = ((0, 512), (512, 512), (1024, 256))
SCALE = 1.0 / float(np.sqrt(np.float32(HD)))

G = 1                       # collective chunks
HG = NH // G                # 8 heads per chunk
PG = 4                      # projection column groups (4 heads each)
PH = NH // PG               # 4 heads per projection group
GC = PH * HD                # 320 feature columns per projection group
VP = HD + 1                 # 81: augmented V row per head [v | 1]
KT_G = HG * HD * SL         # K^T part of one chunk (elems)
V_G = SL * HG * VP          # V part of one chunk
CHUNK = KT_G + V_G          # per-rank chunk elems (bf16)
VROW = HG * VP              # 648: one partition row of the V part


def _build_bass(use_bias):
    import bass_rust
    import concourse.bacc as bacc
    import concourse.tile as tile
    from concourse import mybir
    from concourse.masks import make_identity

    F32 = mybir.dt.float32
    BF16 = mybir.dt.bfloat16
    FP8 = mybir.dt.float8e4
    AF = mybir.ActivationFunctionType
    OP = mybir.AluOpType

    nc = bacc.Bacc("TRN2", target_bir_lowering=False, debug=False,
                   num_devices=NCORES)

    x_io = nc.dram_tensor("x_loc", [SL, H], F32, kind="ExternalInput")
    cos_io = nc.dram_tensor("cosr", [SL, H], F32, kind="ExternalInput")
    sin_io = nc.dram_tensor("sins", [SL, H], F32, kind="ExternalInput")
    wqt_io = nc.dram_tensor("wqt", [H, H], BF16, kind="ExternalInput")
    wkt_io = nc.dram_tensor("wkt", [H, H], BF16, kind="ExternalInput")
    wvt_io = nc.dram_tensor("wvt", [H, H], BF16, kind="ExternalInput")
    wot_io = nc.dram_tensor("wot", [H, H], BF16, kind="ExternalInput")
    w1b_io = nc.dram_tensor("w1b", [MC, H, 128], BF16, kind="ExternalInput")
    w2t_io = nc.dram_tensor("w2t", [MLP, H], BF16, kind="ExternalInput")
    bias5_io = nc.dram_tensor("bias5", [5, H], BF16, kind="ExternalInput")
    b1s_io = nc.dram_tensor("b1s", [128, MC], F32, kind="ExternalInput")
    out_io = nc.dram_tensor("out_loc", [SL, H], F32, kind="ExternalOutput")

    cc_in = [nc.dram_tensor(f"cc_in{g}", [CHUNK], FP8) for g in range(G)]
    cc_out = [nc.dram_tensor(f"cc_out{g}", [NCORES, CHUNK], FP8,
                             addr_space="Shared") for g in range(G)]

    with tile.TileContext(nc) as tc:
        _qrr = [nc.sync, nc.gpsimd, nc.scalar]
        _qi = [0]

        def dmaq():
            e = _qrr[_qi[0] % len(_qrr)]
            _qi[0] += 1
            return e

        const = tc.alloc_tile_pool(name="const", bufs=1)
        persist = tc.alloc_tile_pool(name="persist", bufs=1)
        misc = tc.alloc_tile_pool(name="misc", bufs=2)

        ident = const.tile([128, 128], BF16, name="ident", tag="ident")
        make_identity(nc, ident)
        ones_b = const.tile([1, 128], BF16, name="ones_b", tag="ones_b")
        nc.vector.memset(ones_b, 1.0)
        ones_f = const.tile([1, 128], F32, name="ones_f", tag="ones_f")
        nc.vector.memset(ones_f, 1.0)
        eps_t = const.tile([128, 1], F32, name="eps_t", tag="eps_t")
        nc.vector.memset(eps_t, EPS)
        bias_t = []
        if use_bias:
            for bi in range(5):
                bt = const.tile([1, H], BF16, name=f"bias{bi}",
                                tag=f"bias{bi}")
                nc.sync.dma_start(out=bt, in_=bias5_io[bi:bi + 1, :])
                bias_t.append(bt)
        b1s = const.tile([128, MC], F32, name="b1s", tag="b1s")
        nc.sync.dma_start(out=b1s, in_=b1s_io[:, :])

        x_sb = [persist.tile([128, H], F32, name=f"x{sb}", tag=f"x{sb}")
                for sb in range(SB)]
        for sb in range(SB):
            nc.sync.dma_start(out=x_sb[sb], in_=x_io[sb * 128:(sb + 1) * 128, :])
        x2_sb = [persist.tile([128, H], F32, name=f"x2_{sb}", tag=f"x2_{sb}")
                 for sb in range(SB)]
        qt = [persist.tile([HD, SL], FP8, name=f"qt{h}", tag=f"qt{h}")
              for h in range(NH)]
        attnT = [persist.tile([HD, SL], BF16, name=f"attnT{h}", tag=f"attnT{h}")
                 for h in range(NH)]

        def layernorm_bf16(src, dst):
            # dst[sb] = (src[sb] - mean) * rsqrt(var + eps), cast to bf16
            for sb in range(SB):
                stats = misc.tile([128, 5, 6], F32, name=f"lnst{sb}", tag="lnst")
                sv = src[sb].rearrange("p (g d) -> p g d", d=256)
                for g in range(5):
                    nc.vector.bn_stats(out=stats[:, g, :], in_=sv[:, g, :])
                mv = misc.tile([128, 2], F32, name=f"lnmv{sb}", tag="lnmv")
                nc.vector.bn_aggr(out=mv, in_=stats)
                rstd = misc.tile([128, 1], F32, name=f"lnrs{sb}", tag="lnrs")
                nc.scalar.activation(out=rstd, in_=mv[:, 1:2], func=AF.Sqrt,
                                     bias=eps_t)
                nc.vector.reciprocal(out=rstd, in_=rstd)
                nc.vector.tensor_scalar(out=dst[sb], in0=src[sb],
                                        scalar1=mv[:, 0:1], scalar2=rstd,
                                        op0=OP.subtract, op1=OP.mult)

        def transpose_to(src, dst, ps_pool):
            # src: SB tiles [128, H] bf16 -> dst: HC tiles [128, SL] bf16
            for hc in range(HC):
                for sb in range(SB):
                    pt = ps_pool.tile([128, 128], BF16, name="pt", tag="pt")
                    nc.tensor.transpose(pt, src[sb][:, hc * 128:(hc + 1) * 128],
                                        ident)
                    nc.vector.tensor_copy(
                        out=dst[hc][:, sb * 128:(sb + 1) * 128], in_=pt)

        # ================= phase A: LN1, K/V publish + chunked AllGather,
        # Q projection =================================================
        p_ln = tc.alloc_tile_pool(name="p_ln", bufs=1)
        p_qkv = tc.alloc_tile_pool(name="p_qkv", bufs=1)
        wpool = tc.alloc_tile_pool(name="wpool", bufs=1)
        psA_tr = tc.alloc_tile_pool(name="psA_tr", bufs=2, space="PSUM")
        psA_mm = tc.alloc_tile_pool(name="psA_mm", bufs=2, space="PSUM")

        xln = [p_ln.tile([128, H], BF16, name=f"xln{sb}", tag=f"xln{sb}")
               for sb in range(SB)]
        layernorm_bf16(x_sb, xln)
        xlnT = [p_ln.tile([128, SL], BF16, name=f"xlnT{hc}", tag=f"xlnT{hc}")
                for hc in range(HC)]
        transpose_to(xln, xlnT, psA_tr)

        def load_w(w_io, pfx):
            wt = []
            for hc in range(HC):
                w = wpool.tile([128, H], BF16, name=f"{pfx}{hc}",
                               tag=f"{pfx}{hc}")
                dmaq().dma_start(out=w, in_=w_io[hc * 128:(hc + 1) * 128, :])
                wt.append(w)
            return wt

        wk_t = load_w(wkt_io, "wk")
        wv_t = load_w(wvt_io, "wv")
        wq_t = load_w(wqt_io, "wq")

        def project(wt, bias_idx, dst_fn, groups=range(PG)):
            # dst_fn(sb, g) -> AP of shape [128, GC] for output columns
            # [g*GC, (g+1)*GC)
            for g in groups:
                c0 = g * GC
                for sb in range(SB):
                    ps = psA_mm.tile([128, 512], F32, name="mmps", tag="mmps")
                    for hc in range(HC):
                        nc.tensor.matmul(
                            ps[:, 0:GC],
                            lhsT=xlnT[hc][:, sb * 128:(sb + 1) * 128],
                            rhs=wt[hc][:, c0:c0 + GC],
                            start=(hc == 0),
                            stop=(not use_bias and hc == HC - 1))
                    if use_bias:
                        nc.tensor.matmul(
                            ps[:, 0:GC], lhsT=ones_b,
                            rhs=bias_t[bias_idx][:, c0:c0 + GC],
                            start=False, stop=True)
                    dst = dst_fn(sb, g)
                    src = ps[:, 0:GC]
                    if len(dst.shape) == 3:
                        src = src.rearrange("p (h c) -> p h c", c=HD)
                    nc.scalar.copy(out=dst, in_=src)

        cosr = [p_qkv.tile([128, H], F32, name=f"cos{sb}", tag=f"cos{sb}")
                for sb in range(SB)]
        sins = [p_qkv.tile([128, H], F32, name=f"sin{sb}", tag=f"sin{sb}")
                for sb in range(SB)]
        for sb in range(SB):
            nc.sync.dma_start(out=cosr[sb], in_=cos_io[sb * 128:(sb + 1) * 128, :])
            nc.sync.dma_start(out=sins[sb], in_=sin_io[sb * 128:(sb + 1) * 128, :])

        def rope_g(nat, out, g):
            # RoPE on feature columns [g*GC, (g+1)*GC) (4 heads)
            for sb in range(SB):
                tmp = misc.tile([128, GC], F32, name="ropetmp", tag="ropetmp")
                t3 = tmp.rearrange("p (h c) -> p h c", c=HD)
                q3 = nat[sb].rearrange("p (h c) -> p h c", c=HD)[:, 4 * g:4 * g + 4]
                s3 = sins[sb].rearrange("p (h c) -> p h c", c=HD)[:, 4 * g:4 * g + 4]
                o3 = out[sb].rearrange("p (h c) -> p h c", c=HD)[:, 4 * g:4 * g + 4]
                cs = cosr[sb][:, g * GC:(g + 1) * GC].rearrange(
                    "p (h c) -> p h c", c=HD)
                nc.vector.tensor_mul(out=t3[:, :, 0:40], in0=q3[:, :, 40:80],
                                     in1=s3[:, :, 0:40])
                nc.vector.tensor_mul(out=t3[:, :, 40:80], in0=q3[:, :, 0:40],
                                     in1=s3[:, :, 40:80])
                nc.vector.tensor_mul(out=q3, in0=q3, in1=cs)
                nc.vector.tensor_add(out=o3, in0=q3, in1=t3)

        # ---- K/V: project, rope, transpose, publish per projection group;
        # trigger one AllGather per chunk of two projection groups
        knat = [p_qkv.tile([128, H], F32, name=f"kn{sb}", tag=f"kn{sb}")
                for sb in range(SB)]
        krope = [p_qkv.tile([128, H], BF16, name=f"kr{sb}", tag=f"kr{sb}")
                 for sb in range(SB)]
        ktloc = [p_qkv.tile([HD, SL], FP8, name=f"ktl{h}", tag=f"ktl{h}")
                 for h in range(NH)]

        # V publish staging: [128, NH, VP] rows [v | 1], head-major
        vpub = [p_qkv.tile([128, NH, VP], FP8, name=f"vp{sb}", tag=f"vp{sb}")
                for sb in range(SB)]
        for sb in range(SB):
            nc.vector.memset(vpub[sb][:, :, HD:HD + 1], 1.0)

        pub_q = [nc.sync, nc.scalar]

        kt_in = [cc_in[g][0:KT_G].rearrange("(h d s) -> h d s", h=HG, d=HD)
                 for g in range(G)]
        v_in = [cc_in[g][KT_G:CHUNK].rearrange("(p h c) -> p h c", h=HG, c=VP)
                for g in range(G)]

        ag = []
        for pg in range(PG):
            g, po = divmod(pg, PG // G)
            project(wk_t, 1,
                    lambda sb, gg: knat[sb][:, gg * GC:(gg + 1) * GC],
                    groups=[pg])
            rope_g(knat, krope, pg)
            for hl in range(PH):
                h = pg * PH + hl
                for sb in range(SB):
                    ptk = psA_tr.tile([HD, 128], BF16, name="ptk", tag="pt")
                    nc.tensor.transpose(ptk, krope[sb][:, h * HD:(h + 1) * HD],
                                        ident)
                    nc.vector.tensor_copy(
                        out=ktloc[h][:, sb * 128:(sb + 1) * 128], in_=ptk)
                pub_q[hl % 2].dma_start(out=kt_in[g][h - g * HG], in_=ktloc[h])
            # V for this projection group's columns
            project(wv_t, 2,
                    lambda sb, gg: vpub[sb][:, 4 * gg:4 * gg + 4, 0:HD],
                    groups=[pg])
            for sb in range(SB):
                pub_q[sb % 2].dma_start(
                    out=v_in[g][sb * 128:(sb + 1) * 128,
                                po * PH:(po + 1) * PH, :],
                    in_=vpub[sb][:, pg * PH:(pg + 1) * PH, :])
            if po == PG // G - 1:
                cc = nc.gpsimd.collective_compute(
                    "AllGather", OP.bypass,
                    replica_groups=[list(range(NCORES))],
                    ins=[cc_in[g].ap()], outs=[cc_out[g].ap()])
                ag.append(cc)

        # ---- Q: project, rope, transpose per head
        qnat = [p_qkv.tile([128, H], F32, name=f"qn{sb}", tag=f"qn{sb}")
                for sb in range(SB)]
        qrope = [p_qkv.tile([128, H], BF16, name=f"qr{sb}", tag=f"qr{sb}")
                 for sb in range(SB)]
        project(wq_t, 0, lambda sb, g: qnat[sb][:, g * GC:(g + 1) * GC])
        for pg in range(PG):
            rope_g(qnat, qrope, pg)
            for hl in range(PH):
                h = pg * PH + hl
                for sb in range(SB):
                    ptq = psA_tr.tile([HD, 128], BF16, name="ptq", tag="pt")
                    nc.tensor.transpose(ptq, qrope[sb][:, h * HD:(h + 1) * HD],
                                        ident)
                    nc.vector.tensor_copy(
                        out=qt[h][:, sb * 128:(sb + 1) * 128], in_=ptq)

        psA_mm.release()
        psA_tr.release()
        wpool.release()
        p_qkv.release()
        p_ln.release()

        # O-proj weights prefetch (emitted before the attention loop so the
        # loads run during it; reuses the released QKV-weight SBUF zones)
        wop = tc.alloc_tile_pool(name="wop", bufs=1)
        wo_t = []
        for h in range(NH):
            w = wop.tile([HD, H], BF16, name=f"wo{h}", tag=f"wo{h}")
            ld_q0 = nc.sync if h % 2 == 0 else nc.gpsimd
            ld_q0.dma_start(out=w, in_=wot_io[h * HD:(h + 1) * HD, :])
            wo_t.append(w)

        # ================= phase B: attention (per head group) =========
        # psC_mm takes the one spare PSUM bank so the O-proj accumulation
        # matmuls (emitted after the attention loop) can fill PE-idle slots
        # while attention is ACT(exp)-bound; attention PSUM is unchanged.
        psC_mm = tc.alloc_tile_pool(name="psC_mm", bufs=1, space="PSUM")
        katt = tc.alloc_tile_pool(name="katt", bufs=12)
        vatt = tc.alloc_tile_pool(name="vatt", bufs=16)
        eatt = tc.alloc_tile_pool(name="eatt", bufs=3)
        ps_sc = tc.alloc_tile_pool(name="ps_sc", bufs=2, space="PSUM")
        ps_at = tc.alloc_tile_pool(name="ps_at", bufs=2, space="PSUM")
        ps_rb = tc.alloc_tile_pool(name="ps_rb", bufs=1, space="PSUM")

        # keep the Scalar queue free of DMA issue during attention — ACT
        # exp throughput is the attention bottleneck
        ld_q = [nc.sync, nc.gpsimd]
        for g in range(G):
            # gathered K^T per head: [80, 8 ranks, 256]
            def load_kt(hl):
                kt = katt.tile([HD, NCORES, SL], FP8, name=f"kt{g}_{hl}",
                               tag="kt")
                ktg = cc_out[g][:, hl * HD * SL:(hl + 1) * HD * SL]
                ktg = ktg.rearrange("r (d s) -> d r s", d=HD)
                kdma = ld_q[hl % 2].dma_start(out=kt, in_=ktg)
                bass_rust.add_dep_helper(kdma.ins, ag[g].ins,
                                         reason="wait for remote K")
                return kt

            # the first heads' scores need only their own K^T, but the
            # first AV needs ALL 16 V key blocks — so load kt[0:2], then
            # every va, then the remaining kt.
            kt_g = [load_kt(hl) for hl in range(2)]
            # gathered V (augmented, head-major): per key block [128, HG, 81]
            va_g = []
            for kb in range(KB):
                r, lb = divmod(kb, 2)
                va = vatt.tile([128, HG, VP], FP8, name=f"va{g}_{kb}",
                               tag="va")
                vsrc = cc_out[g][r, KT_G + lb * 128 * VROW:
                                 KT_G + (lb + 1) * 128 * VROW]
                vsrc = vsrc.rearrange("(p f) -> p f", f=VROW)
                vdma = ld_q[kb % 2].dma_start(
                    out=va.rearrange("p h c -> p (h c)"), in_=vsrc)
                bass_rust.add_dep_helper(vdma.ins, ag[g].ins,
                                         reason="wait for remote V")
                va_g.append(va)
            kt_g += [load_kt(hl) for hl in range(2, HG)]

            for hl in range(HG):
                h = g * HG + hl
                ktf = kt_g[hl].rearrange("d r s -> d (r s)")
                e_h = eatt.tile([128, KB, SL], FP8, name=f"e{h}", tag="eh")
                for k4 in range(KB // 4):
                    ps = ps_sc.tile([128, 4 * SL], F32, name="scps", tag="scps")
                    for j in range(4):
                        kb = k4 * 4 + j
                        nc.tensor.matmul(ps[:, j * SL:(j + 1) * SL],
                                         lhsT=ktf[:, kb * 128:(kb + 1) * 128],
                                         rhs=qt[h], start=True, stop=True)
                    ev = e_h[:, k4 * 4:(k4 + 1) * 4, :].rearrange(
                        "p a b -> p (a b)")
                    nc.scalar.activation(out=ev, in_=ps, func=AF.Exp,
                                         scale=SCALE)
                pa = ps_at.tile([VP, SL], F32, name="atps", tag="atps")
                for kb in range(KB):
                    nc.tensor.matmul(pa,
                                     lhsT=va_g[kb][:, hl, :],
                                     rhs=e_h[:, kb, :],
                                     start=(kb == 0), stop=(kb == KB - 1))
                # normalizer: row HD of pa holds Z[q]; move it to partition
                # 0 with a tiny DMA, then broadcast 1/Z across partitions
                # via a rank-1 outer product on the PE.
                zsb = misc.tile([VP, SL], F32, name="zsb", tag="zsb")
                nc.vector.tensor_copy(out=zsb[64:VP, :], in_=pa[64:VP, :])
                zrow = misc.tile([1, SL], F32, name="zrow", tag="zrow")
                nc.sync.dma_start(out=zrow, in_=zsb[HD:VP, :])
                nc.vector.reciprocal_approx_fast(out=zrow, in_=zrow)
                rb = ps_rb.tile([128, SL], F32, name="rbps", tag="rbps")
                nc.tensor.matmul(rb, lhsT=ones_f, rhs=zrow, start=True,
                                 stop=True)
                rbs = misc.tile([HD, SL], F32, name="rbs", tag="rbs")
                nc.vector.tensor_copy(out=rbs, in_=rb[0:HD, :])
                nc.vector.tensor_mul(out=attnT[h], in0=pa[0:HD, :], in1=rbs)

        ps_rb.release()
        ps_at.release()
        ps_sc.release()
        eatt.release()
        vatt.release()
        katt.release()

        # ================= phase C: O projection + residual, LN2 =======
        for sb in range(SB):
            for (c0, cn) in NCOLS:
                ps = psC_mm.tile([128, 512], F32, name="mmps", tag="mmps")
                for h in range(NH):
                    nc.tensor.matmul(ps[:, 0:cn],
                                     lhsT=attnT[h][:, sb * 128:(sb + 1) * 128],
                                     rhs=wo_t[h][:, c0:c0 + cn],
                                     start=(h == 0),
                                     stop=(not use_bias and h == NH - 1))
                if use_bias:
                    nc.tensor.matmul(ps[:, 0:cn], lhsT=ones_b,
                                     rhs=bias_t[3][:, c0:c0 + cn],
                                     start=False, stop=True)
                nc.vector.tensor_add(out=x2_sb[sb][:, c0:c0 + cn],
                                     in0=ps[:, 0:cn],
                                     in1=x_sb[sb][:, c0:c0 + cn])

        psC_mm.release()
        wop.release()

        p_ln2 = tc.alloc_tile_pool(name="p_ln2", bufs=1)
        psC_tr = tc.alloc_tile_pool(name="psC_tr", bufs=2, space="PSUM")

        xln2 = [p_ln2.tile([128, H], BF16, name=f"xln2{sb}", tag=f"xln2{sb}")
                for sb in range(SB)]
        layernorm_bf16(x2_sb, xln2)
        xln2T = [p_ln2.tile([128, SL], BF16, name=f"xln2T{hc}", tag=f"xln2T{hc}")
                 for hc in range(HC)]
        transpose_to(xln2, xln2T, psC_tr)

        psC_tr.release()

        # ================= phase D: MLP ================================
        w1p = tc.alloc_tile_pool(name="w1p", bufs=3)
        w2p = tc.alloc_tile_pool(name="w2p", bufs=3)
        gtp = tc.alloc_tile_pool(name="gtp", bufs=1)
        ps_fc1 = tc.alloc_tile_pool(name="ps_fc1", bufs=2, space="PSUM")
        ps_fc2 = tc.alloc_tile_pool(name="ps_fc2", bufs=1, space="PSUM")

        fc2ps = {}
        for sb in range(SB):
            for (c0, cn) in NCOLS:
                fc2ps[(sb, c0)] = ps_fc2.tile([128, 512], F32,
                                              name=f"fc2ps{sb}_{c0}",
                                              tag=f"fc2ps{sb}_{c0}")
        for mb in range(MC):
            w1 = w1p.tile([128, HC, 128], BF16, name=f"w1_{mb}", tag="w1")
            dmaq().dma_start(out=w1, in_=w1b_io[mb].rearrange(
                "(hc p) m -> p hc m", p=128))
            p1 = ps_fc1.tile([128, SL], F32, name="fc1ps", tag="fc1ps")
            for hc in range(HC):
                nc.tensor.matmul(p1, lhsT=w1[:, hc, :], rhs=xln2T[hc],
                                 start=(hc == 0), stop=(hc == HC - 1))
            gt = gtp.tile([128, SL], BF16, name=f"gt{mb}", tag=f"gt{mb}")
            nc.scalar.activation(out=gt, in_=p1, func=AF.Silu,
                                 scale=1.702, bias=b1s[:, mb:mb + 1])
            w2 = w2p.tile([128, H], BF16, name=f"w2_{mb}", tag="w2")
            dmaq().dma_start(out=w2, in_=w2t_io[mb * 128:(mb + 1) * 128, :])
            for sb in range(SB):
                for (c0, cn) in NCOLS:
                    nc.tensor.matmul(fc2ps[(sb, c0)][:, 0:cn],
                                     lhsT=gt[:, sb * 128:(sb + 1) * 128],
                                     rhs=w2[:, c0:c0 + cn],
                                     start=(mb == 0),
                                     stop=(not use_bias and mb == MC - 1))
        outsb = [persist.tile([128, H], F32, name=f"o{sb}", tag=f"o{sb}")
                 for sb in range(SB)]
        for sb in range(SB):
            for (c0, cn) in NCOLS:
                if use_bias:
                    nc.tensor.matmul(fc2ps[(sb, c0)][:, 0:cn], lhsT=ones_b,
                                     rhs=bias_t[4][:, c0:c0 + cn],
                                     start=False, stop=True)
                nc.vector.tensor_add(out=outsb[sb][:, c0:c0 + cn],
                                     in0=fc2ps[(sb, c0)][:, 0:cn],
                                     in1=x2_sb[sb][:, c0:c0 + cn])
            nc.sync.dma_start(out=out_io[sb * 128:(sb + 1) * 128, :],
                              in_=outsb[sb])

        ps_fc2.release()
        ps_fc1.release()
        gtp.release()
        w2p.release()
        w1p.release()
        p_ln2.release()
        misc.release()
        persist.release()
        const.release()

    nc.compile()
    return nc


_NC = {}


def _get_nc(use_bias=False):
    if use_bias not in _NC:
        _NC[use_bias] = _build_bass(use_bias)
    return _NC[use_bias]


def _prep_inputs(hidden_states, cos, sin,
                 ln1_g, ln1_b, ln2_g, ln2_b,
                 Wq, bq, Wk, bk, Wv, bv, Wo, bo,
                 W1, b1, W2, b2):
    f32 = np.float32
    x = np.asarray(hidden_states, f32).reshape(S, H)
    cos = np.asarray(cos, f32)
    sin = np.asarray(sin, f32)
    g1 = np.asarray(ln1_g, f32); be1 = np.asarray(ln1_b, f32)
    g2 = np.asarray(ln2_g, f32); be2 = np.asarray(ln2_b, f32)
    Wq = np.asarray(Wq, f32); Wk = np.asarray(Wk, f32); Wv = np.asarray(Wv, f32)
    Wo = np.asarray(Wo, f32); W1 = np.asarray(W1, f32); W2 = np.asarray(W2, f32)

    # fold LN1 affine into QKV, LN2 affine into fc1 (exact in fp32)
    wqt = (g1[:, None] * Wq.T).astype(BF)
    wkt = (g1[:, None] * Wk.T).astype(BF)
    wvt = (g1[:, None] * Wv.T).astype(BF)
    bq_e = np.asarray(bq, f32) + Wq @ be1
    bk_e = np.asarray(bk, f32) + Wk @ be1
    bv_e = np.asarray(bv, f32) + Wv @ be1
    wot = Wo.T.astype(BF)
    w1t = g2[:, None] * W1.T                       # [H, MLP]
    w1b = np.ascontiguousarray(
        w1t.reshape(H, MC, 128).transpose(1, 0, 2)).astype(BF)
    b1_e = np.asarray(b1, f32) + W1 @ be2
    b1s = np.ascontiguousarray(
        (1.702 * b1_e).reshape(MC, 128).T).astype(f32)  # [128, MC]
    w2t = (W2.T / 1.702).astype(BF)                 # gelu scale folded
    bias5 = np.stack([bq_e, bk_e, bv_e,
                      np.asarray(bo, f32), np.asarray(b2, f32)]).astype(BF)

    cos_rep = np.tile(cos, (1, NH))                 # [S, H]
    sin_sgn = np.concatenate([-sin[:, :40], sin[:, 40:]], axis=1)
    sin_rep = np.tile(sin_sgn, (1, NH))             # [S, H]

    shared = {
        "wqt": wqt, "wkt": wkt, "wvt": wvt, "wot": wot,
        "w1b": w1b, "w2t": w2t, "bias5": bias5, "b1s": b1s,
    }
    in_maps = []
    for c in range(NCORES):
        sl = slice(c * SL, (c + 1) * SL)
        m = dict(shared)
        m["x_loc"] = np.ascontiguousarray(x[sl])
        m["cosr"] = np.ascontiguousarray(cos_rep[sl])
        m["sins"] = np.ascontiguousarray(sin_rep[sl])
        in_maps.append(m)
    return in_maps


def kernel(hidden_states, attention_mask, cos, sin,
           ln1_g, ln1_b, ln2_g, ln2_b,
           Wq, bq, Wk, bk, Wv, bv, Wo, bo,
           W1, b1, W2, b2):
    # attention_mask is all-True for this problem (spec fill: ones); the
    # dense softmax below assumes it.
    from concourse.bass_utils import run_bass_kernel_spmd

    use_bias = any(
        float(np.abs(np.asarray(b, np.float32)).max()) != 0.0
        for b in (bq, bk, bv, bo, b2))
    nc = _get_nc(use_bias)
    in_maps = _prep_inputs(hidden_states, cos, sin,
                           ln1_g, ln1_b, ln2_g, ln2_b,
                           Wq, bq, Wk, bk, Wv, bv, Wo, bo,
                           W1, b1, W2, b2)
    res = run_bass_kernel_spmd(nc, in_maps, core_ids=list(range(NCORES)))
    out = np.concatenate([res.results[c]["out_loc"] for c in range(NCORES)],
                         axis=0)
    return out.reshape(B, S, H).astype(np.float32)
